# revision 37
# baseline (speedup 1.0000x reference)
"""KoLeo loss kernel for Trainium2, 8 NeuronCores (SPMD), fp8 DoubleRow.

reference math:
    x = thought_vectors.reshape(-1, D)          # [N, D], N=8192, D=1024
    xn = x / max(||x||, 1e-12)
    sim = min(xn @ xn.T, 1.0)
    dist = sqrt(2 - 2*sim + 1e-4), diag -> inf
    loss = -mean(log(min_row_dist + 1e-8))

Key reductions:
  * dist is monotone-decreasing in sim, so only the row-max of the Gram
    matrix (diag excluded) is needed.
  * log(d + 1e-8) ~= 0.5*log(d^2) to ~1e-6 abs, so the final pass is a
    single Ln activation on the clamped row-max.

Implementation (SPMD: all cores run the identical program; anything
core-dependent must come from input data or be structurally uniform):
  * Rows sharded across 8 cores (1024 each). Each core normalizes its
    shard scaled by 16, quantizes to fp8e4 (e4m3), transposes to [D, n]
    layout, AllGathers the fp8 transposed shards, then computes Gram
    blocks with DoubleRow fp8 matmuls (2 k-tiles per instruction, 2x PE
    throughput). Gram values are scaled by 256.
  * 18 column chunks of 512: two "own" chunks from the resident SBUF
    tiles (no AllGather dependency, masked diag via a structural -1024
    diagonal tile) are computed first to hide AllGather A; then all 16
    gathered chunks (8 half-A, then 8 half-B; half-A compute hides
    AllGather B). The core's own columns are recomputed unmasked in the
    gathered pass - its self-similarity (~256, scaled) enters the
    chunk-max table exactly once and is removed by taking the SECOND
    largest value per row in the final pass (vector max8 instruction):
    cross similarities are <=~60 for normalized data, so top-1 is
    always self.
  * PSUM drain alternates between a direct DVE f32 row-max and a
    scalar-engine copy to bf16 followed by a 2x-mode DVE row-max, so no
    single engine limits the tensor engine's psum-bank recycle rate.
"""

import numpy as np

_P = 128
_SCALE = 16.0          # fp8 pre-scale; gram values are scaled by 256
_NEG = -1024.0         # structural diag mask, dominates the +-256 range
_EPS_NORM = 1e-12
_EPS_DIST = 1e-4


def _build_program(ncores, NB, D):
    import concourse.bacc as bacc
    import concourse.mybir as mybir
    from concourse.tile import TileContext
    from concourse.masks import make_identity

    f32 = mybir.dt.float32
    bf16 = mybir.dt.bfloat16
    fp8 = mybir.dt.float8e4
    P = _P
    M_TILES = NB // P          # 8
    K_TILES = D // P           # 8
    HB = NB // 2               # 512, also the column-chunk width
    MH = M_TILES // 2
    CHUNK = HB
    NCHUNK = 2 + 2 * ncores    # 2 resident + 16 gathered chunks
    DR = mybir.MatmulPerfMode.DoubleRow

    nc = bacc.Bacc(
        "TRN2", target_bir_lowering=False, debug=False, num_devices=ncores
    )
    xs = nc.dram_tensor("xs", [NB, D], f32, kind="ExternalInput")
    out = nc.dram_tensor("out", [P, M_TILES], f32, kind="ExternalOutput")

    with TileContext(nc) as tc:
        with (
            tc.tile_pool(name="consts", bufs=1) as consts,
            tc.tile_pool(name="dram", bufs=1, space="DRAM") as dram,
            tc.tile_pool(name="small", bufs=4) as small,
        ):
            identity = consts.tile([P, P], bf16)
            make_identity(nc, identity)
            diagneg = consts.tile([P, P], f32)
            nc.gpsimd.memset(diagneg, 0.0)
            nc.gpsimd.affine_select(
                out=diagneg,
                in_=diagneg,
                compare_op=mybir.AluOpType.not_equal,
                fill=_NEG,
                base=0,
                pattern=[[-1, P]],
                channel_multiplier=1,
            )
            # resident transposed fp8 shard halves: [P(d_lo), k(d_hi), n]
            xnT_A = consts.tile([P, K_TILES, HB], fp8)
            xnT_B = consts.tile([P, K_TILES, HB], fp8)
            maxacc = consts.tile([P, M_TILES, NCHUNK], f32)
            top8 = consts.tile([P, M_TILES * 8], f32)
            outt = consts.tile([P, M_TILES], f32)
            bias_log = consts.tile([P, 1], f32)
            nc.vector.memset(bias_log, 2.0 + _EPS_DIST)

            xnT_localA = dram.tile([D, HB], fp8)
            xnT_localB = dram.tile([D, HB], fp8)
            xnT_allA = dram.tile([ncores * D, HB], fp8, addr_space="Shared")
            xnT_allB = dram.tile([ncores * D, HB], fp8, addr_space="Shared")

            # ---- pre-pass: normalize*16, fp8-quantize, transpose, AG ----
            with (
                tc.tile_pool(name="prep", bufs=3) as prep,
                tc.tile_pool(name="ppsum", bufs=2, space="PSUM") as ppsum,
            ):
                for m in range(M_TILES):
                    xt = prep.tile([P, D], f32, tag="xt")
                    nc.sync.dma_start(xt, xs[m * P : (m + 1) * P, :])
                    sq = prep.tile([P, D], bf16, tag="sq")
                    ss = small.tile([P, 1], f32, tag="ss")
                    nc.scalar.activation(
                        sq,
                        xt,
                        mybir.ActivationFunctionType.Square,
                        accum_out=ss,
                    )
                    # norm/16 = sqrt(ss/256); then 16/norm via reciprocal
                    nrm = small.tile([P, 1], f32, tag="nrm")
                    nc.scalar.activation(
                        nrm,
                        ss,
                        mybir.ActivationFunctionType.Sqrt,
                        scale=1.0 / (_SCALE * _SCALE),
                    )
                    nrm2 = small.tile([P, 1], f32, tag="nrm2")
                    nc.vector.tensor_scalar_max(nrm2, nrm, _EPS_NORM)
                    rinv = small.tile([P, 1], f32, tag="rinv")
                    nc.vector.reciprocal(rinv, nrm2)
                    xnb = prep.tile([P, D], bf16, tag="xnb")
                    nc.vector.tensor_scalar_mul(xnb, xt, rinv)
                    pt = ppsum.tile([P, K_TILES, P], bf16, tag="pt")
                    for k in range(K_TILES):
                        nc.tensor.transpose(
                            pt[:, k, :], xnb[:, k * P : (k + 1) * P], identity
                        )
                    # one fp8-converting copy per m-tile
                    xnT_h = xnT_A if m < MH else xnT_B
                    m4 = m % MH
                    nc.vector.tensor_copy(
                        xnT_h[:, :, m4 * P : (m4 + 1) * P], pt
                    )
                    if m == MH - 1:
                        for k in range(K_TILES):
                            nc.sync.dma_start(
                                xnT_localA[k * P : (k + 1) * P, :],
                                xnT_A[:, k, :],
                            )
                        nc.gpsimd.collective_compute(
                            "AllGather",
                            mybir.AluOpType.bypass,
                            replica_groups=[list(range(ncores))],
                            ins=[xnT_localA.opt()],
                            outs=[xnT_allA.opt()],
                        )
                    # AllGather B is staged and triggered from inside the
                    # main loop (behind the AllGather-A-gated first chunk
                    # DMAs on the sync queue) so its wire traffic does not
                    # contend with the critical first gathered chunks.

            # ---- main pass: 18 x 512-col Gram chunks, running row-max ----
            with (
                tc.tile_pool(name="rhsp", bufs=6) as rhsp,
                tc.tile_pool(name="drainp", bufs=6) as drainp,
                tc.tile_pool(name="mpsum", bufs=8, space="PSUM") as mpsum,
            ):
                order = [("own", 0), ("own", 1)]
                order += [(blk, 0) for blk in range(ncores)]
                order += [(blk, 1) for blk in range(ncores)]

                KH = K_TILES // 2
                for idx, (blk, half) in enumerate(order):
                    if idx == 3:
                        for k in range(K_TILES):
                            nc.sync.dma_start(
                                xnT_localB[k * P : (k + 1) * P, :],
                                xnT_B[:, k, :],
                            )
                        nc.gpsimd.collective_compute(
                            "AllGather",
                            mybir.AluOpType.bypass,
                            replica_groups=[list(range(ncores))],
                            ins=[xnT_localB.opt()],
                            outs=[xnT_allB.opt()],
                        )
                    own = blk == "own"
                    if own:
                        rts = [xnT_A if half == 0 else xnT_B] * 2
                        rk0 = [0, KH]
                    else:
                        # two half-K tiles per chunk so the first matmuls
                        # only wait for half the transfer at each
                        # AllGather handoff
                        src = xnT_allA if half == 0 else xnT_allB
                        rts = []
                        for h in range(2):
                            rt = rhsp.tile([P, KH, CHUNK], fp8, tag=f"rhs{h}")
                            nc.sync.dma_start(
                                rt,
                                src[
                                    blk * D + h * KH * P : blk * D
                                    + (h + 1) * KH * P,
                                    :,
                                ].rearrange("(k p) c -> p k c", k=KH, p=P),
                            )
                            rts.append(rt)
                        rk0 = [0, 0]
                    for m in range(M_TILES):
                        lhsT = xnT_A if m < MH else xnT_B
                        m4 = m % MH
                        ps = mpsum.tile([P, CHUNK], f32, tag="ps")
                        for kk in range(K_TILES // 2):
                            h = kk // 2
                            kb = rk0[h] + (kk % 2) * 2
                            nc.tensor.matmul(
                                ps,
                                lhsT[:, 2 * kk : 2 * kk + 2, m4 * P : (m4 + 1) * P],
                                rts[h][:, kb : kb + 2, :],
                                start=(kk == 0),
                                stop=(kk == K_TILES // 2 - 1),
                                perf_mode=DR,
                            )
                        if own and (m // MH) == half:
                            off = m4 * P
                            nc.vector.tensor_add(
                                ps[:, off : off + P], ps[:, off : off + P], diagneg
                            )
                        if (idx * M_TILES + m) % 2 == 0:
                            sb = drainp.tile([P, CHUNK], bf16, tag="sb")
                            nc.scalar.activation(
                                sb, ps, mybir.ActivationFunctionType.Copy
                            )
                            nc.vector.reduce_max(
                                maxacc[:, m, idx : idx + 1],
                                sb,
                                axis=mybir.AxisListType.X,
                            )
                        else:
                            nc.vector.reduce_max(
                                maxacc[:, m, idx : idx + 1],
                                ps,
                                axis=mybir.AxisListType.X,
                            )

                # ---- final: per-row SECOND max over chunks (drops the
                # one self-similarity entry), clamp, 0.5*log(d^2).
                # Emitted inside the main pool scope so the pool-close
                # drains land after the output DMA instead of stalling
                # the last chunk's matmuls.
                for m in range(M_TILES):
                    nc.vector.max(top8[:, m * 8 : (m + 1) * 8], maxacc[:, m, :])
                mxs = small.tile([P, M_TILES], f32, tag="mxs")
                # gather the 2nd-largest (index 1) of each m-tile's top-8
                nc.vector.tensor_copy(
                    mxs, top8.rearrange("p (m e) -> p m e", e=8)[:, :, 1]
                )
                mxc = small.tile([P, M_TILES], f32, tag="mxc")
                nc.vector.tensor_scalar_min(mxc, mxs, _SCALE * _SCALE)
                # ln(2 + eps - 2*sim) = 2*ln(dist); host multiplies 0.5
                nc.scalar.activation(
                    outt,
                    mxc,
                    mybir.ActivationFunctionType.Ln,
                    bias=bias_log,
                    scale=-2.0 / (_SCALE * _SCALE),
                )
                nc.sync.dma_start(out[:, :], outt)

    nc.compile()
    return nc


def _run(thought_vectors, trace=False, tmpdir=None):
    from concourse.bass_utils import run_bass_kernel_spmd

    ncores, NB, D = 8, 1024, 1024
    x = np.ascontiguousarray(
        np.asarray(thought_vectors, dtype=np.float32).reshape(-1, D)
    )
    N = x.shape[0]
    assert N == ncores * NB

    nc = _build_program(ncores, NB, D)

    in_maps = [{"xs": x[c * NB : (c + 1) * NB]} for c in range(ncores)]

    res = run_bass_kernel_spmd(
        nc,
        in_maps,
        core_ids=list(range(ncores)),
        trace=trace,
        tmpdir=tmpdir,
    )

    total = 0.0
    for c in range(ncores):
        total += float(np.asarray(res.results[c]["out"], dtype=np.float64).sum())
    loss = -0.5 * total / N
    return np.float32(loss), res


def kernel(thought_vectors):
    loss, _ = _run(thought_vectors)
    return np.asarray(loss, dtype=np.float32)


# revision 38
# speedup vs baseline: 1.2417x; 1.2417x over previous
"""KoLeo loss kernel for Trainium2, 8 NeuronCores (SPMD), fp8 DoubleRow.

reference math:
    x = thought_vectors.reshape(-1, D)          # [N, D], N=8192, D=1024
    xn = x / max(||x||, 1e-12)
    sim = min(xn @ xn.T, 1.0)
    dist = sqrt(2 - 2*sim + 1e-4), diag -> inf
    loss = -mean(log(min_row_dist + 1e-8))

Key reductions:
  * dist is monotone-decreasing in sim, so only the row-max of the Gram
    matrix (diag excluded) is needed.
  * log(d + 1e-8) ~= 0.5*log(d^2) to ~1e-6 abs, so the final pass is a
    single Ln activation on the clamped row-max.

Implementation (SPMD: all cores run the identical program; anything
core-dependent must come from input data or be structurally uniform):
  * Rows sharded across 8 cores (1024 each). Each core normalizes its
    shard scaled by 16, quantizes to fp8e4 (e4m3), transposes to [D, n]
    layout, AllGathers the fp8 transposed shards, then computes Gram
    blocks with DoubleRow fp8 matmuls (2 k-tiles per instruction, 2x PE
    throughput). Gram values are scaled by 256.
  * 18 column chunks of 512: two "own" chunks from the resident SBUF
    tiles (no AllGather dependency, masked diag via a structural -1024
    diagonal tile) are computed first to hide AllGather A; then all 16
    gathered chunks (8 half-A, then 8 half-B; half-A compute hides
    AllGather B). The core's own columns are recomputed unmasked in the
    gathered pass - its self-similarity (~256, scaled) enters the
    chunk-max table exactly once and is removed by taking the SECOND
    largest value per row in the final pass (vector max8 instruction):
    cross similarities are <=~60 for normalized data, so top-1 is
    always self.
  * PSUM drain alternates between a direct DVE f32 row-max and a
    scalar-engine copy to bf16 followed by a 2x-mode DVE row-max, so no
    single engine limits the tensor engine's psum-bank recycle rate.
"""

import numpy as np

_P = 128
_SCALE = 16.0          # fp8 pre-scale; gram values are scaled by 256
_NEG = -1024.0         # structural diag mask, dominates the +-256 range
_EPS_NORM = 1e-12
_EPS_DIST = 1e-4


def _build_program(ncores, NB, D):
    import concourse.bacc as bacc
    import concourse.mybir as mybir
    from concourse.tile import TileContext
    from concourse.masks import make_identity

    f32 = mybir.dt.float32
    bf16 = mybir.dt.bfloat16
    fp8 = mybir.dt.float8e4
    P = _P
    M_TILES = NB // P          # 8
    K_TILES = D // P           # 8
    HB = NB // 2               # 512, also the column-chunk width
    MH = M_TILES // 2
    CHUNK = HB
    NCHUNK = 2 + 2 * ncores    # 2 resident + 16 gathered chunks
    DR = mybir.MatmulPerfMode.DoubleRow

    nc = bacc.Bacc(
        "TRN2", target_bir_lowering=False, debug=False, num_devices=ncores
    )
    xs = nc.dram_tensor("xs", [NB, D], f32, kind="ExternalInput")
    out = nc.dram_tensor("out", [P, M_TILES], f32, kind="ExternalOutput")

    with TileContext(nc) as tc:
        with (
            tc.tile_pool(name="consts", bufs=1) as consts,
            tc.tile_pool(name="dram", bufs=1, space="DRAM") as dram,
            tc.tile_pool(name="small", bufs=4) as small,
        ):
            identity = consts.tile([P, P], bf16)
            make_identity(nc, identity)
            diagneg = consts.tile([P, P], f32)
            nc.gpsimd.memset(diagneg, 0.0)
            nc.gpsimd.affine_select(
                out=diagneg,
                in_=diagneg,
                compare_op=mybir.AluOpType.not_equal,
                fill=_NEG,
                base=0,
                pattern=[[-1, P]],
                channel_multiplier=1,
            )
            # resident transposed fp8 shard halves: [P(d_lo), k(d_hi), n]
            xnT_A = consts.tile([P, K_TILES, HB], fp8)
            xnT_B = consts.tile([P, K_TILES, HB], fp8)
            maxacc = consts.tile([P, M_TILES, NCHUNK], f32)
            top8 = consts.tile([P, M_TILES * 8], f32)
            outt = consts.tile([P, M_TILES], f32)
            bias_log = consts.tile([P, 1], f32)
            nc.vector.memset(bias_log, 2.0 + _EPS_DIST)

            xnT_localA = dram.tile([D, HB], fp8)
            xnT_localB = dram.tile([D, HB], fp8)
            xnT_allA = dram.tile([ncores * D, HB], fp8, addr_space="Shared")
            xnT_allB = dram.tile([ncores * D, HB], fp8, addr_space="Shared")

            # ---- pre-pass: normalize*16, fp8-quantize, transpose, AG ----
            with (
                tc.tile_pool(name="prep", bufs=3) as prep,
                tc.tile_pool(name="ppsum", bufs=2, space="PSUM") as ppsum,
            ):
                for m in range(M_TILES):
                    xt = prep.tile([P, D], f32, tag="xt")
                    nc.sync.dma_start(xt, xs[m * P : (m + 1) * P, :])
                    sq = prep.tile([P, D], bf16, tag="sq")
                    ss = small.tile([P, 1], f32, tag="ss")
                    nc.scalar.activation(
                        sq,
                        xt,
                        mybir.ActivationFunctionType.Square,
                        accum_out=ss,
                    )
                    # norm/16 = sqrt(ss/256); then 16/norm via reciprocal
                    nrm = small.tile([P, 1], f32, tag="nrm")
                    nc.scalar.activation(
                        nrm,
                        ss,
                        mybir.ActivationFunctionType.Sqrt,
                        scale=1.0 / (_SCALE * _SCALE),
                    )
                    nrm2 = small.tile([P, 1], f32, tag="nrm2")
                    nc.vector.tensor_scalar_max(nrm2, nrm, _EPS_NORM)
                    rinv = small.tile([P, 1], f32, tag="rinv")
                    nc.vector.reciprocal(rinv, nrm2)
                    xnb = prep.tile([P, D], bf16, tag="xnb")
                    nc.vector.tensor_scalar_mul(xnb, xt, rinv)
                    pt = ppsum.tile([P, K_TILES, P], bf16, tag="pt")
                    for k in range(K_TILES):
                        nc.tensor.transpose(
                            pt[:, k, :], xnb[:, k * P : (k + 1) * P], identity
                        )
                    # one fp8-converting copy per m-tile
                    xnT_h = xnT_A if m < MH else xnT_B
                    m4 = m % MH
                    nc.vector.tensor_copy(
                        xnT_h[:, :, m4 * P : (m4 + 1) * P], pt
                    )
                    if m == MH - 1:
                        for k in range(K_TILES):
                            nc.sync.dma_start(
                                xnT_localA[k * P : (k + 1) * P, :],
                                xnT_A[:, k, :],
                            )
                        nc.gpsimd.collective_compute(
                            "AllGather",
                            mybir.AluOpType.bypass,
                            replica_groups=[list(range(ncores))],
                            ins=[xnT_localA.opt()],
                            outs=[xnT_allA.opt()],
                        )
                    elif m == M_TILES - 1:
                        for k in range(K_TILES):
                            nc.sync.dma_start(
                                xnT_localB[k * P : (k + 1) * P, :],
                                xnT_B[:, k, :],
                            )
                        nc.gpsimd.collective_compute(
                            "AllGather",
                            mybir.AluOpType.bypass,
                            replica_groups=[list(range(ncores))],
                            ins=[xnT_localB.opt()],
                            outs=[xnT_allB.opt()],
                        )

            # ---- main pass: 18 x 512-col Gram chunks, running row-max ----
            with (
                tc.tile_pool(name="rhsp", bufs=6) as rhsp,
                tc.tile_pool(name="drainp", bufs=6) as drainp,
                tc.tile_pool(name="mpsum", bufs=8, space="PSUM") as mpsum,
            ):
                order = [("own", 0), ("own", 1)]
                order += [(blk, 0) for blk in range(ncores)]
                order += [(blk, 1) for blk in range(ncores)]

                KH = K_TILES // 2
                for idx, (blk, half) in enumerate(order):
                    own = blk == "own"
                    if own:
                        rts = [xnT_A if half == 0 else xnT_B] * 2
                        rk0 = [0, KH]
                    else:
                        # two half-K tiles per chunk so the first matmuls
                        # only wait for half the transfer at each
                        # AllGather handoff
                        src = xnT_allA if half == 0 else xnT_allB
                        rts = []
                        for h in range(2):
                            rt = rhsp.tile([P, KH, CHUNK], fp8, tag=f"rhs{h}")
                            nc.sync.dma_start(
                                rt,
                                src[
                                    blk * D + h * KH * P : blk * D
                                    + (h + 1) * KH * P,
                                    :,
                                ].rearrange("(k p) c -> p k c", k=KH, p=P),
                            )
                            rts.append(rt)
                        rk0 = [0, 0]
                    for m in range(M_TILES):
                        lhsT = xnT_A if m < MH else xnT_B
                        m4 = m % MH
                        ps = mpsum.tile([P, CHUNK], f32, tag="ps")
                        for kk in range(K_TILES // 2):
                            h = kk // 2
                            kb = rk0[h] + (kk % 2) * 2
                            nc.tensor.matmul(
                                ps,
                                lhsT[:, 2 * kk : 2 * kk + 2, m4 * P : (m4 + 1) * P],
                                rts[h][:, kb : kb + 2, :],
                                start=(kk == 0),
                                stop=(kk == K_TILES // 2 - 1),
                                perf_mode=DR,
                            )
                        if own and (m // MH) == half:
                            off = m4 * P
                            nc.vector.tensor_add(
                                ps[:, off : off + P], ps[:, off : off + P], diagneg
                            )
                        if (idx * M_TILES + m) % 2 == 0:
                            sb = drainp.tile([P, CHUNK], bf16, tag="sb")
                            nc.scalar.activation(
                                sb, ps, mybir.ActivationFunctionType.Copy
                            )
                            nc.vector.reduce_max(
                                maxacc[:, m, idx : idx + 1],
                                sb,
                                axis=mybir.AxisListType.X,
                            )
                        else:
                            nc.vector.reduce_max(
                                maxacc[:, m, idx : idx + 1],
                                ps,
                                axis=mybir.AxisListType.X,
                            )

                # ---- final: per-row SECOND max over chunks (drops the
                # one self-similarity entry), clamp, 0.5*log(d^2).
                # Emitted inside the main pool scope so the pool-close
                # drains land after the output DMA instead of stalling
                # the last chunk's matmuls.
                for m in range(M_TILES):
                    nc.vector.max(top8[:, m * 8 : (m + 1) * 8], maxacc[:, m, :])
                mxs = small.tile([P, M_TILES], f32, tag="mxs")
                # gather the 2nd-largest (index 1) of each m-tile's top-8
                nc.vector.tensor_copy(
                    mxs, top8.rearrange("p (m e) -> p m e", e=8)[:, :, 1]
                )
                mxc = small.tile([P, M_TILES], f32, tag="mxc")
                nc.vector.tensor_scalar_min(mxc, mxs, _SCALE * _SCALE)
                # ln(2 + eps - 2*sim) = 2*ln(dist); host multiplies 0.5
                nc.scalar.activation(
                    outt,
                    mxc,
                    mybir.ActivationFunctionType.Ln,
                    bias=bias_log,
                    scale=-2.0 / (_SCALE * _SCALE),
                )
                nc.sync.dma_start(out[:, :], outt)

    nc.compile()
    return nc


def _run(thought_vectors, trace=False, tmpdir=None):
    from concourse.bass_utils import run_bass_kernel_spmd

    ncores, NB, D = 8, 1024, 1024
    x = np.ascontiguousarray(
        np.asarray(thought_vectors, dtype=np.float32).reshape(-1, D)
    )
    N = x.shape[0]
    assert N == ncores * NB

    nc = _build_program(ncores, NB, D)

    in_maps = [{"xs": x[c * NB : (c + 1) * NB]} for c in range(ncores)]

    res = run_bass_kernel_spmd(
        nc,
        in_maps,
        core_ids=list(range(ncores)),
        trace=trace,
        tmpdir=tmpdir,
    )

    total = 0.0
    for c in range(ncores):
        total += float(np.asarray(res.results[c]["out"], dtype=np.float64).sum())
    loss = -0.5 * total / N
    return np.float32(loss), res


def kernel(thought_vectors):
    loss, _ = _run(thought_vectors)
    return np.asarray(loss, dtype=np.float32)
